# revision 6
# baseline (speedup 1.0000x reference)
"""CRF NLL kernel for Trainium2 (8 NeuronCores, batch-sharded).

Math (validated in numpy): forward algorithm in exp-space with a constant
per-step normalizer C, split bidirectionally (fwd from s=0, bwd from s=1023)
to halve the serial recursion depth. Both chains run fused in one
128-partition pipeline: partitions 0-63 carry the fwd state, 64-127 the bwd
state; one block-diagonal 128x128 stationary matmul + one [128,64] vector
multiply per slot (511 slots).

Score side: emission gather via iota==tag one-hot multiply-accumulate on
GPSIMD (reads emissions in natural layout); transition/start/end terms are
dot products of host-side integer tag bincounts with the parameter tensors
(device does all float math touching parameters).

Output: per-core partial sums [1,8]; host combines and takes the mean.
"""

import numpy as np

S, B, T, NCORES = 1024, 512, 64, 8
BC = B // NCORES          # 64 batch per core
NSLOT = (S - 2) // 2      # 511 recursion slots per chain
CH_E = 7                  # slots per e-chunk (73 * 7 = 511)
NCH_E = NSLOT // CH_E     # 73
NTILE = S * BC // 128     # 512 natural-layout tiles of 128 rows
CH_N = 8                  # natural tiles per chunk
NCH_N = NTILE // CH_N     # 64
CNORM = 4.66              # constant per-step normalizer (log-space)

_COMPILED = None


def _build_program():
    from contextlib import ExitStack

    import concourse.bacc as bacc
    import concourse.tile as tile
    import concourse.mybir as mybir

    f32 = mybir.dt.float32
    Exp = mybir.ActivationFunctionType.Exp
    Log = mybir.ActivationFunctionType.Ln
    mult = mybir.AluOpType.mult
    add = mybir.AluOpType.add
    is_equal = mybir.AluOpType.is_equal
    AX = mybir.AxisListType

    nc = bacc.Bacc(
        "TRN2",
        target_bir_lowering=False,
        debug=False,
        enable_asserts=False,
        num_devices=NCORES,
    )

    def din(name, shape):
        return nc.dram_tensor(name, shape, f32, kind="ExternalInput").ap()

    em_stack = din("em_stack", [NCH_E, 128, CH_E * T])    # exp-chain input, slot-stacked
    em_init = din("em_init", [128, BC])                   # em[0].T / em[1023].T
    em_nat = din("em_nat", [NCH_N, 128, CH_N * T])        # natural layout chunks
    tags_t = din("tags_t", [128, NTILE])                  # tags transposed (f32)
    iota64 = din("iota64", [128, T])                      # row of 0..63 per partition
    trans2 = din("trans2", [128, T])                      # [trans ; trans^T] stacked
    cpair = din("cpair", [T, T])                          # pair bincount (f32)
    cse = din("cse", [128, 1])                            # [count_start ; count_end]
    pse = din("pse", [128, 1])                            # [start_transitions ; end_transitions]
    out_part = nc.dram_tensor("out_part", [1, 8], f32, kind="ExternalOutput").ap()

    with tile.TileContext(nc) as tc, ExitStack() as ctx:
        const = ctx.enter_context(tc.tile_pool(name="const", bufs=1))
        eraw_p = ctx.enter_context(tc.tile_pool(name="eraw", bufs=3))
        eexp_p = ctx.enter_context(tc.tile_pool(name="eexp", bufs=3))
        nat_p = ctx.enter_context(tc.tile_pool(name="nat", bufs=3))
        alpha_p = ctx.enter_context(tc.tile_pool(name="alpha", bufs=3))
        scr_p = ctx.enter_context(tc.tile_pool(name="scr", bufs=2))
        small_p = ctx.enter_context(tc.tile_pool(name="small", bufs=1))
        psum_p = ctx.enter_context(tc.tile_pool(name="psum", bufs=2, space="PSUM"))
        psumf_p = ctx.enter_context(tc.tile_pool(name="psumf", bufs=1, space="PSUM"))

        # ---- constants into SBUF
        iota_sb = const.tile([128, T], f32)
        nc.sync.dma_start(iota_sb[:], iota64)
        tags_sb = const.tile([128, NTILE], f32)
        nc.sync.dma_start(tags_sb[:], tags_t)
        tt_sb = const.tile([128, T], f32)
        nc.sync.dma_start(tt_sb[:], trans2)
        cpair_sb = const.tile([T, T], f32)
        nc.sync.dma_start(cpair_sb[:], cpair)
        cse_sb = const.tile([128, 1], f32)
        nc.sync.dma_start(cse_sb[:], cse)
        pse_sb = const.tile([128, 1], f32)
        nc.sync.dma_start(pse_sb[:], pse)
        eminit_sb = const.tile([128, BC], f32)
        nc.sync.dma_start(eminit_sb[:], em_init)

        # ---- stationary weights: W = [[expT, 0], [0, expT^T]], W2 = [[0],[expT^T]]
        W = const.tile([128, 128], f32)
        nc.vector.memset(W[:], 0.0)
        nc.scalar.activation(W[0:64, 0:64], tt_sb[0:64, :], Exp)
        nc.scalar.activation(W[64:128, 64:128], tt_sb[64:128, :], Exp)
        W2 = const.tile([128, T], f32)
        nc.vector.memset(W2[:], 0.0)
        nc.scalar.activation(W2[64:128, :], tt_sb[64:128, :], Exp)

        # ---- init state: alpha0 = exp(em_init + [start; end - C])
        negc_col = const.tile([128, 1], f32)
        nc.vector.memset(negc_col[:], -CNORM)
        bias_col = const.tile([128, 1], f32)
        nc.vector.tensor_copy(bias_col[0:64, :], pse_sb[0:64, :])
        nc.vector.tensor_scalar_add(bias_col[64:128, :], pse_sb[64:128, :], -CNORM)
        alpha = alpha_p.tile([128, BC], f32)
        nc.scalar.activation(alpha[:], eminit_sb[:], Exp, bias=bias_col[:, 0:1])

        # ---- emission accumulator (written by GPSIMD, one column per tile)
        emit_acc = const.tile([128, NTILE], f32)

        # ---- bidirectional recursion chain, with one emission-gather op per
        # slot interleaved on DVE (fills the chain's dependency-latency gaps)
        nat = None

        def emit_tile(ti):
            nonlocal nat
            if ti % CH_N == 0:
                nat = nat_p.tile([128, CH_N * T], f32)
                nc.sync.dma_start(nat[:], em_nat[ti // CH_N])
            c = ti % CH_N
            scratch = scr_p.tile([128, T], f32)
            nc.vector.scalar_tensor_tensor(
                scratch[:],
                iota_sb[:],
                tags_sb[:, ti:ti + 1],
                nat[:, c * T:(c + 1) * T],
                op0=is_equal,
                op1=mult,
                accum_out=emit_acc[:, ti:ti + 1],
            )

        for g in range(NCH_E):
            raw = eraw_p.tile([128, CH_E * T], f32)
            nc.sync.dma_start(raw[:], em_stack[g])
            e = eexp_p.tile([128, CH_E * T], f32)
            nc.scalar.activation(e[:], raw[:], Exp, bias=negc_col[:, 0:1])
            for k in range(CH_E):
                gamma = psum_p.tile([128, BC], f32)
                nc.tensor.matmul(gamma[:], W[:], alpha[:], start=True, stop=True)
                alpha = alpha_p.tile([128, BC], f32)
                nc.vector.tensor_mul(alpha[:], gamma[:], e[:, k * T:(k + 1) * T])
                emit_tile(g * CH_E + k)
        emit_tile(NTILE - 1)

        # ---- join: Z_b = sum_i alpha_fwd[i,b] * (expT @ bt)[i,b]
        gfin = psumf_p.tile([64, BC], f32)
        nc.tensor.matmul(gfin[:], W2[:], alpha[:], start=True, stop=True)
        zprod = small_p.tile([64, BC], f32)
        nc.vector.tensor_mul(zprod[:], gfin[:], alpha[0:64, :])
        zcol = small_p.tile([1, BC], f32)
        nc.gpsimd.tensor_reduce(zcol[:], zprod[:], axis=AX.C, op=add)
        logz = small_p.tile([1, BC], f32)
        nc.scalar.activation(logz[:], zcol[:], Log)
        logz_sum = small_p.tile([1, 1], f32)
        nc.vector.tensor_reduce(logz_sum[:], logz[:], axis=AX.X, op=add)

        # ---- score dot products
        emit_col = small_p.tile([128, 1], f32)
        nc.vector.tensor_reduce(emit_col[:], emit_acc[:], axis=AX.X, op=add)
        emit_sum = small_p.tile([1, 1], f32)
        nc.gpsimd.tensor_reduce(emit_sum[:], emit_col[:], axis=AX.C, op=add)

        tscr = small_p.tile([T, T], f32)
        td_col = small_p.tile([T, 1], f32)
        nc.vector.scalar_tensor_tensor(
            tscr[:], cpair_sb[:], 1.0, tt_sb[0:64, :],
            op0=mult, op1=mult, accum_out=td_col[:],
        )
        trans_dot = small_p.tile([1, 1], f32)
        nc.gpsimd.tensor_reduce(trans_dot[:], td_col[:], axis=AX.C, op=add)

        se_col = small_p.tile([128, 1], f32)
        nc.vector.tensor_mul(se_col[:], cse_sb[:], pse_sb[:])
        se_sum = small_p.tile([1, 1], f32)
        nc.gpsimd.tensor_reduce(se_sum[:], se_col[:], axis=AX.C, op=add)

        # ---- ship partials
        nc.sync.dma_start(out_part[0:1, 0:1], logz_sum[:])
        nc.sync.dma_start(out_part[0:1, 1:2], emit_sum[:])
        nc.sync.dma_start(out_part[0:1, 2:3], trans_dot[:])
        nc.sync.dma_start(out_part[0:1, 3:4], se_sum[:])

    nc.compile()
    return nc


def _get_compiled():
    global _COMPILED
    if _COMPILED is None:
        _COMPILED = _build_program()
    return _COMPILED


def _prep_core(em_c, tags_c, trans, start, end, iota_arr):
    """Build the per-core input map (numpy only; index/layout prep + bincounts)."""
    emT = np.ascontiguousarray(em_c.transpose(0, 2, 1))      # [S, T, BC]
    stack = np.empty((NSLOT, 128, BC), np.float32)
    stack[:, :64, :] = emT[1:1 + NSLOT]                       # fwd: em[1..511]
    stack[:, 64:, :] = emT[S - 2:S - 2 - NSLOT:-1]            # bwd: em[1022..512]
    em_stack = np.ascontiguousarray(
        stack.reshape(NCH_E, CH_E, 128, BC).transpose(0, 2, 1, 3).reshape(NCH_E, 128, CH_E * T)
    )
    em_init = np.concatenate([emT[0], emT[S - 1]], axis=0).astype(np.float32)  # [128, BC]
    nat = np.ascontiguousarray(em_c.reshape(NTILE, 128, T))
    em_nat = np.ascontiguousarray(
        nat.reshape(NCH_N, CH_N, 128, T).transpose(0, 2, 1, 3).reshape(NCH_N, 128, CH_N * T)
    )
    tflat = tags_c.reshape(-1).astype(np.int64)
    tags_t = np.ascontiguousarray(
        tflat.reshape(NTILE, 128).T.astype(np.float32)
    )
    cpair = np.bincount(
        (tags_c[:-1].astype(np.int64) * T + tags_c[1:]).reshape(-1), minlength=T * T
    ).reshape(T, T).astype(np.float32)
    cs = np.bincount(tags_c[0], minlength=T).astype(np.float32)
    ce = np.bincount(tags_c[-1], minlength=T).astype(np.float32)
    trans2 = np.concatenate([trans, trans.T], axis=0).astype(np.float32)  # [128, T]
    cse = np.concatenate([cs, ce]).reshape(128, 1).astype(np.float32)
    pse = np.concatenate([start, end]).reshape(128, 1).astype(np.float32)
    return {
        "em_stack": em_stack,
        "em_init": em_init,
        "em_nat": em_nat,
        "tags_t": tags_t,
        "iota64": iota_arr,
        "trans2": trans2,
        "cpair": cpair,
        "cse": cse,
        "pse": pse,
    }


def kernel(emissions, tags, mask, transitions, start_transitions, end_transitions,
           _trace=False):
    from concourse.bass_utils import run_bass_kernel_spmd

    em = np.asarray(emissions, np.float32)
    tg = np.asarray(tags)
    tr = np.asarray(transitions, np.float32)
    st = np.asarray(start_transitions, np.float32)
    en = np.asarray(end_transitions, np.float32)
    # mask is all-ones in this problem setup; sequence lengths are full.

    iota_arr = np.tile(np.arange(T, dtype=np.float32), (128, 1))
    in_maps = []
    for c in range(NCORES):
        sl = slice(c * BC, (c + 1) * BC)
        in_maps.append(_prep_core(
            np.ascontiguousarray(em[:, sl, :]),
            np.ascontiguousarray(tg[:, sl]).astype(np.int64),
            tr, st, en, iota_arr,
        ))

    nc = _get_compiled()
    res = run_bass_kernel_spmd(nc, in_maps, core_ids=list(range(NCORES)),
                               trace=_trace)
    total = 0.0
    for c in range(NCORES):
        p = res.results[c]["out_part"].reshape(-1).astype(np.float64)
        logz_sum, emit_sum, trans_dot, se_sum = p[0], p[1], p[2], p[3]
        logz_sum += BC * (S - 1) * CNORM
        total += logz_sum - (emit_sum + trans_dot + se_sum)
    out = np.float32(total / B)
    if _trace:
        return out, res
    return out


# revision 7
# speedup vs baseline: 201.2018x; 201.2018x over previous
"""CRF NLL kernel for Trainium2 (8 NeuronCores, batch-sharded).

Math (validated in numpy): forward algorithm in exp-space with a constant
per-step normalizer C, split bidirectionally (fwd from s=0, bwd from s=1023)
to halve the serial recursion depth. Both chains run fused in one
128-partition pipeline: partitions 0-63 carry the fwd state, 64-127 the bwd
state; one block-diagonal 128x128 stationary matmul + one [128,64] vector
multiply per slot (511 slots).

Score side: emission gather via iota==tag one-hot multiply-accumulate on
GPSIMD (reads emissions in natural layout); transition/start/end terms are
dot products of host-side integer tag bincounts with the parameter tensors
(device does all float math touching parameters).

Output: per-core partial sums [1,8]; host combines and takes the mean.
"""

import numpy as np

S, B, T, NCORES = 1024, 512, 64, 8
BC = B // NCORES          # 64 batch per core
NSLOT = (S - 2) // 2      # 511 recursion slots per chain
CH_E = 7                  # slots per e-chunk (73 * 7 = 511)
NCH_E = NSLOT // CH_E     # 73
NTILE = S * BC // 128     # 512 natural-layout tiles of 128 rows
CH_N = 8                  # natural tiles per chunk
NCH_N = NTILE // CH_N     # 64
CNORM = 4.66              # constant per-step normalizer (log-space)

_COMPILED = {}


def _build_program(repeat=1):
    from contextlib import ExitStack

    import concourse.bacc as bacc
    import concourse.tile as tile
    import concourse.mybir as mybir

    f32 = mybir.dt.float32
    Exp = mybir.ActivationFunctionType.Exp
    Log = mybir.ActivationFunctionType.Ln
    mult = mybir.AluOpType.mult
    add = mybir.AluOpType.add
    is_equal = mybir.AluOpType.is_equal
    AX = mybir.AxisListType

    nc = bacc.Bacc(
        "TRN2",
        target_bir_lowering=False,
        debug=False,
        enable_asserts=False,
        num_devices=NCORES,
    )

    def din(name, shape):
        return nc.dram_tensor(name, shape, f32, kind="ExternalInput").ap()

    em_stack = din("em_stack", [NCH_E, 128, CH_E * T])    # exp-chain input, slot-stacked
    em_init = din("em_init", [128, BC])                   # em[0].T / em[1023].T
    em_nat = din("em_nat", [NCH_N, 128, CH_N * T])        # natural layout chunks
    tags_t = din("tags_t", [128, NTILE])                  # tags transposed (f32)
    iota64 = din("iota64", [128, T])                      # row of 0..63 per partition
    trans2 = din("trans2", [128, T])                      # [trans ; trans^T] stacked
    cpair = din("cpair", [T, T])                          # pair bincount (f32)
    cse = din("cse", [128, 1])                            # [count_start ; count_end]
    pse = din("pse", [128, 1])                            # [start_transitions ; end_transitions]
    out_part = nc.dram_tensor("out_part", [1, 8], f32, kind="ExternalOutput").ap()

    with tile.TileContext(nc) as tc, ExitStack() as ctx:
        const = ctx.enter_context(tc.tile_pool(name="const", bufs=1))
        eraw_p = ctx.enter_context(tc.tile_pool(name="eraw", bufs=3))
        eexp_p = ctx.enter_context(tc.tile_pool(name="eexp", bufs=3))
        nat_p = ctx.enter_context(tc.tile_pool(name="nat", bufs=3))
        alpha_p = ctx.enter_context(tc.tile_pool(name="alpha", bufs=3))
        scr_p = ctx.enter_context(tc.tile_pool(name="scr", bufs=2))
        small_p = ctx.enter_context(tc.tile_pool(name="small", bufs=1))
        psum_p = ctx.enter_context(tc.tile_pool(name="psum", bufs=2, space="PSUM"))
        psumf_p = ctx.enter_context(tc.tile_pool(name="psumf", bufs=1, space="PSUM"))

        # ---- constants into SBUF
        iota_sb = const.tile([128, T], f32)
        nc.sync.dma_start(iota_sb[:], iota64)
        tags_sb = const.tile([128, NTILE], f32)
        nc.sync.dma_start(tags_sb[:], tags_t)
        tt_sb = const.tile([128, T], f32)
        nc.sync.dma_start(tt_sb[:], trans2)
        cpair_sb = const.tile([T, T], f32)
        nc.sync.dma_start(cpair_sb[:], cpair)
        cse_sb = const.tile([128, 1], f32)
        nc.sync.dma_start(cse_sb[:], cse)
        pse_sb = const.tile([128, 1], f32)
        nc.sync.dma_start(pse_sb[:], pse)
        eminit_sb = const.tile([128, BC], f32)
        nc.sync.dma_start(eminit_sb[:], em_init)

        # ---- stationary weights: W = [[expT, 0], [0, expT^T]], W2 = [[0],[expT^T]]
        W = const.tile([128, 128], f32)
        nc.vector.memset(W[:], 0.0)
        nc.scalar.activation(W[0:64, 0:64], tt_sb[0:64, :], Exp)
        nc.scalar.activation(W[64:128, 64:128], tt_sb[64:128, :], Exp)
        W2 = const.tile([128, T], f32)
        nc.vector.memset(W2[:], 0.0)
        nc.scalar.activation(W2[64:128, :], tt_sb[64:128, :], Exp)

        # ---- init state: alpha0 = exp(em_init + [start; end - C])
        negc_col = const.tile([128, 1], f32)
        nc.vector.memset(negc_col[:], -CNORM)
        bias_col = const.tile([128, 1], f32)
        nc.vector.tensor_copy(bias_col[0:64, :], pse_sb[0:64, :])
        nc.vector.tensor_scalar_add(bias_col[64:128, :], pse_sb[64:128, :], -CNORM)
        alpha = alpha_p.tile([128, BC], f32)
        nc.scalar.activation(alpha[:], eminit_sb[:], Exp, bias=bias_col[:, 0:1])

        # ---- emission accumulator (written by GPSIMD, one column per tile)
        emit_acc = const.tile([128, NTILE], f32)

        # ---- bidirectional recursion chain, with one emission-gather op per
        # slot interleaved on DVE (fills the chain's dependency-latency gaps)
        import contextlib
        rep_ctx = tc.For_i(0, repeat, 1) if repeat > 1 else contextlib.nullcontext()
        ctx.enter_context(rep_ctx)
        if repeat > 1:
            alpha = alpha_p.tile([128, BC], f32)
            nc.scalar.activation(alpha[:], eminit_sb[:], Exp, bias=bias_col[:, 0:1])
        nat = None

        def emit_tile(ti):
            nonlocal nat
            if ti % CH_N == 0:
                nat = nat_p.tile([128, CH_N * T], f32)
                nc.sync.dma_start(nat[:], em_nat[ti // CH_N])
            c = ti % CH_N
            scratch = scr_p.tile([128, T], f32)
            nc.vector.scalar_tensor_tensor(
                scratch[:],
                iota_sb[:],
                tags_sb[:, ti:ti + 1],
                nat[:, c * T:(c + 1) * T],
                op0=is_equal,
                op1=mult,
                accum_out=emit_acc[:, ti:ti + 1],
            )

        for g in range(NCH_E):
            raw = eraw_p.tile([128, CH_E * T], f32)
            nc.sync.dma_start(raw[:], em_stack[g])
            e = eexp_p.tile([128, CH_E * T], f32)
            nc.scalar.activation(e[:], raw[:], Exp, bias=negc_col[:, 0:1])
            for k in range(CH_E):
                gamma = psum_p.tile([128, BC], f32)
                nc.tensor.matmul(gamma[:], W[:], alpha[:], start=True, stop=True)
                alpha = alpha_p.tile([128, BC], f32)
                nc.vector.tensor_mul(alpha[:], gamma[:], e[:, k * T:(k + 1) * T])
                emit_tile(g * CH_E + k)
        emit_tile(NTILE - 1)

        # ---- join: Z_b = sum_i alpha_fwd[i,b] * (expT @ bt)[i,b]
        gfin = psumf_p.tile([64, BC], f32)
        nc.tensor.matmul(gfin[:], W2[:], alpha[:], start=True, stop=True)
        zprod = small_p.tile([64, BC], f32)
        nc.vector.tensor_mul(zprod[:], gfin[:], alpha[0:64, :])
        zcol = small_p.tile([1, BC], f32)
        nc.gpsimd.tensor_reduce(zcol[:], zprod[:], axis=AX.C, op=add)
        logz = small_p.tile([1, BC], f32)
        nc.scalar.activation(logz[:], zcol[:], Log)
        logz_sum = small_p.tile([1, 1], f32)
        nc.vector.tensor_reduce(logz_sum[:], logz[:], axis=AX.X, op=add)

        # ---- score dot products
        emit_col = small_p.tile([128, 1], f32)
        nc.vector.tensor_reduce(emit_col[:], emit_acc[:], axis=AX.X, op=add)
        emit_sum = small_p.tile([1, 1], f32)
        nc.gpsimd.tensor_reduce(emit_sum[:], emit_col[:], axis=AX.C, op=add)

        tscr = small_p.tile([T, T], f32)
        td_col = small_p.tile([T, 1], f32)
        nc.vector.scalar_tensor_tensor(
            tscr[:], cpair_sb[:], 1.0, tt_sb[0:64, :],
            op0=mult, op1=mult, accum_out=td_col[:],
        )
        trans_dot = small_p.tile([1, 1], f32)
        nc.gpsimd.tensor_reduce(trans_dot[:], td_col[:], axis=AX.C, op=add)

        se_col = small_p.tile([128, 1], f32)
        nc.vector.tensor_mul(se_col[:], cse_sb[:], pse_sb[:])
        se_sum = small_p.tile([1, 1], f32)
        nc.gpsimd.tensor_reduce(se_sum[:], se_col[:], axis=AX.C, op=add)

        # ---- ship partials
        nc.sync.dma_start(out_part[0:1, 0:1], logz_sum[:])
        nc.sync.dma_start(out_part[0:1, 1:2], emit_sum[:])
        nc.sync.dma_start(out_part[0:1, 2:3], trans_dot[:])
        nc.sync.dma_start(out_part[0:1, 3:4], se_sum[:])

    nc.compile()
    return nc


def _get_compiled(repeat=1):
    if repeat not in _COMPILED:
        _COMPILED[repeat] = _build_program(repeat)
    return _COMPILED[repeat]


def _prep_core(em_c, tags_c, trans, start, end, iota_arr):
    """Build the per-core input map (numpy only; index/layout prep + bincounts)."""
    emT = np.ascontiguousarray(em_c.transpose(0, 2, 1))      # [S, T, BC]
    stack = np.empty((NSLOT, 128, BC), np.float32)
    stack[:, :64, :] = emT[1:1 + NSLOT]                       # fwd: em[1..511]
    stack[:, 64:, :] = emT[S - 2:S - 2 - NSLOT:-1]            # bwd: em[1022..512]
    em_stack = np.ascontiguousarray(
        stack.reshape(NCH_E, CH_E, 128, BC).transpose(0, 2, 1, 3).reshape(NCH_E, 128, CH_E * T)
    )
    em_init = np.concatenate([emT[0], emT[S - 1]], axis=0).astype(np.float32)  # [128, BC]
    nat = np.ascontiguousarray(em_c.reshape(NTILE, 128, T))
    em_nat = np.ascontiguousarray(
        nat.reshape(NCH_N, CH_N, 128, T).transpose(0, 2, 1, 3).reshape(NCH_N, 128, CH_N * T)
    )
    tflat = tags_c.reshape(-1).astype(np.int64)
    tags_t = np.ascontiguousarray(
        tflat.reshape(NTILE, 128).T.astype(np.float32)
    )
    cpair = np.bincount(
        (tags_c[:-1].astype(np.int64) * T + tags_c[1:]).reshape(-1), minlength=T * T
    ).reshape(T, T).astype(np.float32)
    cs = np.bincount(tags_c[0], minlength=T).astype(np.float32)
    ce = np.bincount(tags_c[-1], minlength=T).astype(np.float32)
    trans2 = np.concatenate([trans, trans.T], axis=0).astype(np.float32)  # [128, T]
    cse = np.concatenate([cs, ce]).reshape(128, 1).astype(np.float32)
    pse = np.concatenate([start, end]).reshape(128, 1).astype(np.float32)
    return {
        "em_stack": em_stack,
        "em_init": em_init,
        "em_nat": em_nat,
        "tags_t": tags_t,
        "iota64": iota_arr,
        "trans2": trans2,
        "cpair": cpair,
        "cse": cse,
        "pse": pse,
    }


def kernel(emissions, tags, mask, transitions, start_transitions, end_transitions,
           _trace=False):
    from concourse.bass_utils import run_bass_kernel_spmd

    em = np.asarray(emissions, np.float32)
    tg = np.asarray(tags)
    tr = np.asarray(transitions, np.float32)
    st = np.asarray(start_transitions, np.float32)
    en = np.asarray(end_transitions, np.float32)
    # mask is all-ones in this problem setup; sequence lengths are full.

    iota_arr = np.tile(np.arange(T, dtype=np.float32), (128, 1))
    in_maps = []
    for c in range(NCORES):
        sl = slice(c * BC, (c + 1) * BC)
        in_maps.append(_prep_core(
            np.ascontiguousarray(em[:, sl, :]),
            np.ascontiguousarray(tg[:, sl]).astype(np.int64),
            tr, st, en, iota_arr,
        ))

    nc = _get_compiled()
    res = run_bass_kernel_spmd(nc, in_maps, core_ids=list(range(NCORES)),
                               trace=_trace)
    total = 0.0
    for c in range(NCORES):
        p = res.results[c]["out_part"].reshape(-1).astype(np.float64)
        logz_sum, emit_sum, trans_dot, se_sum = p[0], p[1], p[2], p[3]
        logz_sum += BC * (S - 1) * CNORM
        total += logz_sum - (emit_sum + trans_dot + se_sum)
    out = np.float32(total / B)
    if _trace:
        return out, res
    return out
